# revision 24
# baseline (speedup 1.0000x reference)
"""Conv2d 3x3 VALID stride-1 kernel for Trainium2 (Bass/Tile), 8-core SPMD.

x: [32, 128, 112, 112] f32, weight: [256, 128, 3, 3] f32
out: [32, 256, 110, 110] f32

Strategy: implicit GEMM, data-parallel over batch (4 images/core).
Cin=128 sits on the SBUF partition dim and is the matmul contraction
axis; output row-chunks of <=4 rows (free dim <=448 <= 512 fp32 = one
PSUM bank) accumulate the 9 filter taps in PSUM.

MIXED-PRECISION TAP SPLIT: 7 of the 9 taps run as fp16 matmuls
(1 row/cycle); taps (kh=0,kw=0) and (kh=1,kw=0) run TOGETHER as a single
fp8-e4m3 `perf_mode=DoubleRow` matmul, which virtualizes the PE array to
128x256 and contracts both taps in one pass (~1.13x the cycles of ONE
fp16 tap instead of two). The DoubleRow moving operand is an overlapping
4D access pattern [Cin, pair=2 (stride W), rows (stride W), cols] on an
fp8 copy of x; the stationary operand packs the two taps' weights as
[Cin, 2, 128]. This cuts PE work ~10% (363 -> ~331 us roofline).
Accuracy: quantizing 2/9 of the contraction to e4m3 gives rel_fro
~1.77e-2 on this randn data (measured numerically; gate 2e-2) --
fp16-only is 2.9e-4, full-fp8 would be 3.75e-2.

All inputs are pre-cast on the HOST (x to fp16 + e4m3, weights packed
per cout-tile) so every DMA is cast-free and can issue from any
engine's queues; head-critical pieces are spread across the Sync /
Scalar / GpSimd queue sets in need-order. Dependency-free N=128 warm-up
matmuls run from the framework preamble (~6 us) to data-ready (~10.5 us)
so the PE HAM clock gate flips to 2.4 GHz on garbage work.
"""

import numpy as np

import concourse.mybir as mybir
import concourse.tile as tile
from concourse import bacc
from concourse.ap import AP
from concourse.bass_utils import run_bass_kernel_spmd

B, CIN, H, W = 32, 128, 112, 112
COUT, KH, KW = 256, 3, 3
OH, OW = H - KH + 1, W - KW + 1  # 110, 110
NCORES = 8
BPC = B // NCORES  # batches per core

F32 = mybir.dt.float32
FP16 = mybir.dt.float16
FP8 = mybir.dt.float8e4
COMPUTE_DT = FP16

# The two taps fused into the fp8 DoubleRow matmul (rows kh=0,1 at kw=0:
# pair stride = W bytes = 112, 16B-aligned at every chunk offset) and the
# seven that stay fp16.
TAPS8 = [(0, 0), (1, 0)]
TAPS16 = [(kh, kw) for kh in range(KH) for kw in range(KW)
          if (kh, kw) not in TAPS8]

# Row-chunking of the 110 output rows: free dim = rows*OW <= 512 (PSUM
# bank). First chunk R=3 so the first matmul group depends on only 5
# input rows; last chunk R=3 keeps the serial output tail small.
ROW_CHUNKS = [3] + [4] * 26 + [3]

_CACHE = {}


def _build_nc():
    nc = bacc.Bacc("TRN2", target_bir_lowering=False, debug=False)

    x_d = nc.dram_tensor("x", [BPC, CIN, H, W], FP16, kind="ExternalInput")
    # fp8 pair-interleaved x: x8p[ci, r, c, :] = (x[r, c], x[r+1, c]) so
    # the DoubleRow moving operand reads byte-adjacent pairs (16-bit
    # lane reads, 2 fp8/cycle). A strided pair streams serially: measured
    # 417 ns/MM vs 186 ns/MM packed.
    HP = OH + 1  # 111 pair-rows
    x8_d = nc.dram_tensor("x8", [BPC, CIN, HP, W, 2], FP8,
                          kind="ExternalInput")
    # fp16 weights packed [ci, ct, tap7, co128] (cout-tile OUTER so each
    # cout-tile's load is one contiguous per-partition run; slicing a
    # [ci, tap, co256] layout shatters the DMA into 256 B packets).
    w_d = nc.dram_tensor("w", [CIN, 2, len(TAPS16), 128], FP16,
                         kind="ExternalInput")
    # fp8 pair weights [ci, ct, 2, co128]
    w8_d = nc.dram_tensor("w8", [CIN, 2, 2, 128], FP8, kind="ExternalInput")
    o_d = nc.dram_tensor("o", [BPC, COUT, OH, OW], F32,
                         kind="ExternalOutput")

    from concourse.bass import _add_dep_helper

    # Prefetch chunking of images b >= 1, paced against the previous
    # batch's compute so the input stream never bursts hard enough to
    # starve the output stores of SDMA bandwidth.
    PF_BOUNDS = [0, 14, 28, 42, 56, 70, 84, 98, 112]
    N_GROUPS = 2 * len(ROW_CHUNKS)  # (row-chunk, ct) groups per batch

    with tile.TileContext(nc) as tc:
        with (
            tc.tile_pool(name="wpool", bufs=1) as wpool,
            tc.tile_pool(name="xpool", bufs=3) as xpool,
            tc.tile_pool(name="x8pool", bufs=3) as x8pool,
            tc.tile_pool(name="opool", bufs=16) as opool,
            tc.tile_pool(name="psum", bufs=8, space="PSUM") as psum,
        ):
            # PE pre-warm: dependency-light dummy matmuls on a small
            # scratch tile keep the PE busy from engine boot until the
            # first real matmul's data arrives, so the HAM clock gate
            # (free-running 3.4 us busy window) flips to 2.4 GHz on
            # garbage work before the real stream begins.
            scratch = wpool.tile([128, 128], FP16, name="warm_scratch")
            nc.vector.memset(scratch[:], 0)
            ps_warm = psum.tile([128, 128], F32, name="warm_psum", tag="ps")
            for _ in range(40):
                nc.tensor.matmul(
                    ps_warm[:], scratch[:], scratch[:],
                    start=True, stop=True, skip_group_check=True,
                )

            wr = wpool.tile([CIN, 2, len(TAPS16), 128], FP16)
            w8r = wpool.tile([CIN, 2, 2, 128], FP8)
            xtiles = [xpool.tile([CIN, H, W], FP16, tag="x", name="x0")]
            x8tiles = [x8pool.tile([CIN, HP, W, 2], FP8, tag="x8",
                                   name="x8_0")]

            # Head-critical loads spread across FOUR engine queue sets so
            # their ~0.65us DMA-instruction executions and transfers all
            # run in parallel. Group 0 needs x8 pair-rows 0:3 + w8 (the
            # DoubleRow matmul leads each group) + x rows 0:5 + w[ct0];
            # ct1 is needed ~1.7 us after T0.
            # w[ct0] as ONE transfer (1792 B per-partition runs) leading
            # the fast Sync queue: tap-wise splits shrank the runs to
            # 256-768 B and crawled (~115 KB/us measured).
            nc.sync.dma_start(wr[:, 0], w_d[:, 0])
            nc.sync.dma_start(xtiles[0][:, 0:3, :], x_d[0, :, 0:3, :])
            nc.sync.dma_start(xtiles[0][:, 3:5, :], x_d[0, :, 3:5, :])
            nc.sync.dma_start(xtiles[0][:, 5:12, :], x_d[0, :, 5:12, :])

            nc.scalar.dma_start(w8r[:], w8_d[:, :, :, :])
            nc.scalar.dma_start(wr[:, 1], w_d[:, 1])
            nc.gpsimd.dma_start(x8tiles[0][:, 0:6], x8_d[0, :, 0:6])
            nc.gpsimd.dma_start(x8tiles[0][:, 6:16], x8_d[0, :, 6:16])
            for a, b_, is8 in [(12, 20, 0), (16, 30, 1), (20, 35, 0),
                               (30, 50, 1), (35, 50, 0), (50, 70, 1),
                               (50, 70, 0), (70, 90, 1), (70, 90, 0),
                               (90, 112, 1), (90, 112, 0)]:
                if is8:
                    b_ = min(b_, HP)
                    nc.scalar.dma_start(
                        x8tiles[0][:, a:b_], x8_d[0, :, a:b_]
                    )
                else:
                    nc.scalar.dma_start(
                        xtiles[0][:, a:b_, :], x_d[0, :, a:b_, :]
                    )

            for b in range(BPC):
                xr = xtiles[b]
                x8r = x8tiles[b]
                if b + 1 < BPC:
                    xtiles.append(
                        xpool.tile([CIN, H, W], FP16, tag="x",
                                   name=f"x{b+1}")
                    )
                    x8tiles.append(
                        x8pool.tile([CIN, HP, W, 2], FP8, tag="x8",
                                    name=f"x8_{b+1}")
                    )
                # Milestone group index at which to release prefetch chunk
                # j of image b+1: spread the 8 chunk-pairs across this
                # batch's 56 groups.
                pf_at = {
                    (N_GROUPS * j) // len(PF_BOUNDS[1:]): j
                    for j in range(len(PF_BOUNDS) - 1)
                }

                # Interleave the two cout-tiles per row-chunk: halves the
                # x-row consumption rate so compute never overruns the
                # image DMA at kernel start.
                oh = 0
                gidx = 0
                for R in ROW_CHUNKS:
                    for ct in range(2):
                        co0 = ct * 128
                        ps = psum.tile([128, R, OW], F32, tag="ps")
                        for idx, (kh, kw) in enumerate(TAPS16):
                            nc.tensor.matmul(
                                ps[:],
                                wr[:, ct, idx, :],
                                xr[:, oh + kh : oh + kh + R, kw : kw + OW],
                                start=(idx == 0),
                                stop=False,
                            )
                        # fp8 DoubleRow last: both TAPS8 taps in one
                        # matmul. Moving operand = packed-pair 4D AP
                        # [Cin, pair=2 (step 1), R (stride 2W), OW (step
                        # 2)] on the host-interleaved fp8 x.
                        rhs8 = AP(
                            tensor=x8r[:].tensor,
                            offset=oh * W * 2,
                            ap=[[HP * W * 2, 128], [1, 2],
                                [2 * W, R], [2, OW]],
                        )
                        nc.tensor.matmul(
                            ps[:], w8r[:, ct], rhs8,
                            start=False, stop=True,
                            perf_mode=mybir.MatmulPerfMode.DoubleRow,
                        )
                        ot = opool.tile([128, R, OW], F32, tag="ot")
                        cp = nc.vector.tensor_copy(ot[:], ps[:])
                        is_last = (b == BPC - 1 and gidx == N_GROUPS - 1)
                        if is_last:
                            # Split the final store by partition halves
                            # across two queue sets: halves the per-engine
                            # packet count on the serial tail after the
                            # last matmul.
                            nc.sync.dma_start(
                                o_d[b, co0 : co0 + 64, oh : oh + R, :],
                                ot[0:64],
                            )
                            nc.scalar.dma_start(
                                o_d[b, co0 + 64 : co0 + 128, oh : oh + R, :],
                                ot[64:128],
                            )
                        else:
                            # Alternate output stores between the Sync and
                            # Scalar queue sets.
                            oeng = nc.sync if gidx % 2 == 0 else nc.scalar
                            oeng.dma_start(
                                o_d[b, co0 : co0 + 128, oh : oh + R, :],
                                ot[:],
                            )
                        if b == 0 and gidx < 2:
                            # Anti-cascade wedge: a few dependency-free
                            # matmuls right after the first two groups
                            # keep the PE busy if the early input DMAs
                            # jitter late, so a short data gap cannot
                            # idle the PE past the HAM MID window and
                            # re-throttle the clock (observed: one 2.5us
                            # early stall cascading into ~14us of
                            # half-clock matmuls).
                            for _ in range(5):
                                nc.tensor.matmul(
                                    ps_warm[:], scratch[:], scratch[:],
                                    start=True, stop=True,
                                    skip_group_check=True,
                                )
                        if b + 1 < BPC and gidx in pf_at:
                            j = pf_at[gidx]
                            r0, r1 = PF_BOUNDS[j], PF_BOUNDS[j + 1]
                            dma = nc.gpsimd.dma_start(
                                xtiles[b + 1][:, r0:r1, :],
                                x_d[b + 1, :, r0:r1, :],
                            )
                            _add_dep_helper(
                                dma.ins,
                                cp.ins,
                                sync=True,
                                reason="pace input prefetch vs compute",
                            )
                            dma8 = nc.gpsimd.dma_start(
                                x8tiles[b + 1][:, r0 : min(r1, HP)],
                                x8_d[b + 1, :, r0 : min(r1, HP)],
                            )
                            _add_dep_helper(
                                dma8.ins,
                                cp.ins,
                                sync=True,
                                reason="pace fp8 input prefetch vs compute",
                            )
                        gidx += 1
                    oh += R

    nc.compile()
    return nc


def _get_nc():
    if "nc" not in _CACHE:
        _CACHE["nc"] = _build_nc()
    return _CACHE["nc"]


LAST_RESULT = None


def kernel(x, weight, trace=False):
    global LAST_RESULT
    import ml_dtypes

    x = np.ascontiguousarray(np.asarray(x, dtype=np.float32))
    # Host-side casts: fp16 for the 7 plain taps, e4m3 for the DoubleRow
    # pair. (Values here are well within TRN e4m3's +-240 range, where
    # the OCP e4m3fn bit patterns match TRN FP8_EXP4.)
    x16 = x.astype(np.float16)
    x8 = x.astype(ml_dtypes.float8_e4m3fn)
    # pair-interleave: x8p[..., r, c, i] = x8[..., r+i, c]
    x8p = np.ascontiguousarray(
        np.stack([x8[:, :, 0 : H - 1, :], x8[:, :, 1:H, :]], axis=-1)
    )

    weight = np.asarray(weight, dtype=np.float32)
    wct = weight.reshape(2, 128, CIN, KH, KW)  # [ct, co128, ci, kh, kw]
    # fp16 taps packed [ci, ct, tap7, co128]
    w16 = np.ascontiguousarray(
        np.stack([wct[:, :, :, kh, kw] for (kh, kw) in TAPS16], axis=0)
        .transpose(3, 1, 0, 2)
    ).astype(np.float16)
    # fp8 pair packed [ci, ct, 2, co128]
    w8 = np.ascontiguousarray(
        np.stack([wct[:, :, :, kh, kw] for (kh, kw) in TAPS8], axis=0)
        .transpose(3, 1, 0, 2)
    ).astype(ml_dtypes.float8_e4m3fn)

    nc = _get_nc()
    in_maps = [
        {
            "x": x16[i * BPC : (i + 1) * BPC],
            "x8": x8p[i * BPC : (i + 1) * BPC],
            "w": w16,
            "w8": w8,
        }
        for i in range(NCORES)
    ]
    res = run_bass_kernel_spmd(
        nc, in_maps, core_ids=list(range(NCORES)), trace=trace
    )
    LAST_RESULT = res
    out = np.concatenate([r["o"] for r in res.results], axis=0)
    return out


# revision 25
# speedup vs baseline: 1.0089x; 1.0089x over previous
"""Conv2d 3x3 VALID stride-1 kernel for Trainium2 (Bass/Tile), 8-core SPMD.

x: [32, 128, 112, 112] f32, weight: [256, 128, 3, 3] f32
out: [32, 256, 110, 110] f32

Strategy: implicit GEMM, data-parallel over batch (4 images/core).
Cin=128 sits on the SBUF partition dim and is the matmul contraction
axis; output row-chunks of <=4 rows (free dim <=448 <= 512 fp32 = one
PSUM bank) accumulate the 9 filter taps in PSUM.

MIXED-PRECISION TAP SPLIT: 7 of the 9 taps run as fp16 matmuls
(1 row/cycle); taps (kh=0,kw=0) and (kh=1,kw=0) run TOGETHER as a single
fp8-e4m3 `perf_mode=DoubleRow` matmul, which virtualizes the PE array to
128x256 and contracts both taps in one pass (~1.13x the cycles of ONE
fp16 tap instead of two). The DoubleRow moving operand is an overlapping
4D access pattern [Cin, pair=2 (stride W), rows (stride W), cols] on an
fp8 copy of x; the stationary operand packs the two taps' weights as
[Cin, 2, 128]. This cuts PE work ~10% (363 -> ~331 us roofline).
Accuracy: quantizing 2/9 of the contraction to e4m3 gives rel_fro
~1.77e-2 on this randn data (measured numerically; gate 2e-2) --
fp16-only is 2.9e-4, full-fp8 would be 3.75e-2.

All inputs are pre-cast on the HOST (x to fp16 + e4m3, weights packed
per cout-tile) so every DMA is cast-free and can issue from any
engine's queues; head-critical pieces are spread across the Sync /
Scalar / GpSimd queue sets in need-order. Dependency-free N=128 warm-up
matmuls run from the framework preamble (~6 us) to data-ready (~10.5 us)
so the PE HAM clock gate flips to 2.4 GHz on garbage work.
"""

import numpy as np

import concourse.mybir as mybir
import concourse.tile as tile
from concourse import bacc
from concourse.ap import AP
from concourse.bass_utils import run_bass_kernel_spmd

B, CIN, H, W = 32, 128, 112, 112
COUT, KH, KW = 256, 3, 3
OH, OW = H - KH + 1, W - KW + 1  # 110, 110
NCORES = 8
BPC = B // NCORES  # batches per core

F32 = mybir.dt.float32
FP16 = mybir.dt.float16
FP8 = mybir.dt.float8e4
COMPUTE_DT = FP16

# The two taps fused into the fp8 DoubleRow matmul (rows kh=0,1 at kw=0:
# pair stride = W bytes = 112, 16B-aligned at every chunk offset) and the
# seven that stay fp16.
TAPS8 = [(0, 0), (1, 0)]
TAPS16 = [(kh, kw) for kh in range(KH) for kw in range(KW)
          if (kh, kw) not in TAPS8]

# Row-chunking of the 110 output rows: free dim = rows*OW <= 512 (PSUM
# bank). First chunk R=3 so the first matmul group depends on only 5
# input rows; last chunk R=3 keeps the serial output tail small.
ROW_CHUNKS = [3] + [4] * 26 + [3]

_CACHE = {}


def _build_nc():
    nc = bacc.Bacc("TRN2", target_bir_lowering=False, debug=False)

    x_d = nc.dram_tensor("x", [BPC, CIN, H, W], FP16, kind="ExternalInput")
    # fp8 pair-interleaved x: x8p[ci, r, c, :] = (x[r, c], x[r+1, c]) so
    # the DoubleRow moving operand reads byte-adjacent pairs (16-bit
    # lane reads, 2 fp8/cycle). A strided pair streams serially: measured
    # 417 ns/MM vs 186 ns/MM packed.
    HP = OH + 1  # 111 pair-rows
    x8_d = nc.dram_tensor("x8", [BPC, CIN, HP, W, 2], FP8,
                          kind="ExternalInput")
    # fp16 weights packed [ci, ct, tap7, co128] (cout-tile OUTER so each
    # cout-tile's load is one contiguous per-partition run; slicing a
    # [ci, tap, co256] layout shatters the DMA into 256 B packets).
    w_d = nc.dram_tensor("w", [CIN, 2, len(TAPS16), 128], FP16,
                         kind="ExternalInput")
    # fp8 pair weights [ci, ct, 2, co128]
    w8_d = nc.dram_tensor("w8", [CIN, 2, 2, 128], FP8, kind="ExternalInput")
    o_d = nc.dram_tensor("o", [BPC, COUT, OH, OW], F32,
                         kind="ExternalOutput")

    from concourse.bass import _add_dep_helper

    # Prefetch chunking of images b >= 1, paced against the previous
    # batch's compute so the input stream never bursts hard enough to
    # starve the output stores of SDMA bandwidth.
    PF_BOUNDS = [0, 14, 28, 42, 56, 70, 84, 98, 112]
    N_GROUPS = 2 * len(ROW_CHUNKS)  # (row-chunk, ct) groups per batch

    with tile.TileContext(nc) as tc:
        with (
            tc.tile_pool(name="wpool", bufs=1) as wpool,
            tc.tile_pool(name="xpool", bufs=3) as xpool,
            tc.tile_pool(name="x8pool", bufs=3) as x8pool,
            tc.tile_pool(name="opool", bufs=16) as opool,
            tc.tile_pool(name="psum", bufs=8, space="PSUM") as psum,
        ):
            # PE pre-warm: dependency-light dummy matmuls on a small
            # scratch tile keep the PE busy from engine boot until the
            # first real matmul's data arrives, so the HAM clock gate
            # (free-running 3.4 us busy window) flips to 2.4 GHz on
            # garbage work before the real stream begins.
            scratch = wpool.tile([128, 128], FP16, name="warm_scratch")
            nc.vector.memset(scratch[:], 0)
            ps_warm = psum.tile([128, 128], F32, name="warm_psum", tag="ps")
            for _ in range(44):
                nc.tensor.matmul(
                    ps_warm[:], scratch[:], scratch[:],
                    start=True, stop=True, skip_group_check=True,
                )

            wr = wpool.tile([CIN, 2, len(TAPS16), 128], FP16)
            w8r = wpool.tile([CIN, 2, 2, 128], FP8)
            xtiles = [xpool.tile([CIN, H, W], FP16, tag="x", name="x0")]
            x8tiles = [x8pool.tile([CIN, HP, W, 2], FP8, tag="x8",
                                   name="x8_0")]

            # Head-critical loads spread across FOUR engine queue sets so
            # their ~0.65us DMA-instruction executions and transfers all
            # run in parallel. Group 0 needs x8 pair-rows 0:3 + w8 (the
            # DoubleRow matmul leads each group) + x rows 0:5 + w[ct0];
            # ct1 is needed ~1.7 us after T0.
            nc.sync.dma_start(xtiles[0][:, 0:3, :], x_d[0, :, 0:3, :])
            nc.sync.dma_start(xtiles[0][:, 3:5, :], x_d[0, :, 3:5, :])
            nc.sync.dma_start(xtiles[0][:, 5:12, :], x_d[0, :, 5:12, :])

            nc.scalar.dma_start(wr[:, 0, 0:1, :], w_d[:, 0, 0:1, :])
            nc.scalar.dma_start(wr[:, 0, 1:4, :], w_d[:, 0, 1:4, :])
            nc.scalar.dma_start(wr[:, 0, 4:7, :], w_d[:, 0, 4:7, :])
            nc.scalar.dma_start(w8r[:], w8_d[:, :, :, :])
            nc.scalar.dma_start(wr[:, 1], w_d[:, 1])
            nc.gpsimd.dma_start(x8tiles[0][:, 0:6], x8_d[0, :, 0:6])
            nc.gpsimd.dma_start(x8tiles[0][:, 6:16], x8_d[0, :, 6:16])
            for a, b_, is8 in [(12, 20, 0), (16, 30, 1), (20, 35, 0),
                               (30, 50, 1), (35, 50, 0), (50, 70, 1),
                               (50, 70, 0), (70, 90, 1), (70, 90, 0),
                               (90, 112, 1), (90, 112, 0)]:
                if is8:
                    b_ = min(b_, HP)
                    nc.scalar.dma_start(
                        x8tiles[0][:, a:b_], x8_d[0, :, a:b_]
                    )
                else:
                    nc.scalar.dma_start(
                        xtiles[0][:, a:b_, :], x_d[0, :, a:b_, :]
                    )

            for b in range(BPC):
                xr = xtiles[b]
                x8r = x8tiles[b]
                if b + 1 < BPC:
                    xtiles.append(
                        xpool.tile([CIN, H, W], FP16, tag="x",
                                   name=f"x{b+1}")
                    )
                    x8tiles.append(
                        x8pool.tile([CIN, HP, W, 2], FP8, tag="x8",
                                    name=f"x8_{b+1}")
                    )
                # Milestone group index at which to release prefetch chunk
                # j of image b+1: spread the 8 chunk-pairs across this
                # batch's 56 groups.
                pf_at = {
                    (N_GROUPS * j) // len(PF_BOUNDS[1:]): j
                    for j in range(len(PF_BOUNDS) - 1)
                }

                # Interleave the two cout-tiles per row-chunk: halves the
                # x-row consumption rate so compute never overruns the
                # image DMA at kernel start.
                oh = 0
                gidx = 0
                for R in ROW_CHUNKS:
                    for ct in range(2):
                        co0 = ct * 128
                        ps = psum.tile([128, R, OW], F32, tag="ps")
                        for idx, (kh, kw) in enumerate(TAPS16):
                            nc.tensor.matmul(
                                ps[:],
                                wr[:, ct, idx, :],
                                xr[:, oh + kh : oh + kh + R, kw : kw + OW],
                                start=(idx == 0),
                                stop=False,
                            )
                        # fp8 DoubleRow last: both TAPS8 taps in one
                        # matmul. Moving operand = packed-pair 4D AP
                        # [Cin, pair=2 (step 1), R (stride 2W), OW (step
                        # 2)] on the host-interleaved fp8 x.
                        rhs8 = AP(
                            tensor=x8r[:].tensor,
                            offset=oh * W * 2,
                            ap=[[HP * W * 2, 128], [1, 2],
                                [2 * W, R], [2, OW]],
                        )
                        nc.tensor.matmul(
                            ps[:], w8r[:, ct], rhs8,
                            start=False, stop=True,
                            perf_mode=mybir.MatmulPerfMode.DoubleRow,
                        )
                        ot = opool.tile([128, R, OW], F32, tag="ot")
                        cp = nc.vector.tensor_copy(ot[:], ps[:])
                        is_last = (b == BPC - 1 and gidx == N_GROUPS - 1)
                        if is_last:
                            # Split the final store by partition halves
                            # across two queue sets: halves the per-engine
                            # packet count on the serial tail after the
                            # last matmul.
                            nc.sync.dma_start(
                                o_d[b, co0 : co0 + 64, oh : oh + R, :],
                                ot[0:64],
                            )
                            nc.scalar.dma_start(
                                o_d[b, co0 + 64 : co0 + 128, oh : oh + R, :],
                                ot[64:128],
                            )
                        else:
                            # Alternate output stores between the Sync and
                            # Scalar queue sets.
                            oeng = nc.sync if gidx % 2 == 0 else nc.scalar
                            oeng.dma_start(
                                o_d[b, co0 : co0 + 128, oh : oh + R, :],
                                ot[:],
                            )
                        if b == 0 and gidx < 2:
                            # Anti-cascade wedge: a few dependency-free
                            # matmuls right after the first two groups
                            # keep the PE busy if the early input DMAs
                            # jitter late, so a short data gap cannot
                            # idle the PE past the HAM MID window and
                            # re-throttle the clock (observed: one 2.5us
                            # early stall cascading into ~14us of
                            # half-clock matmuls).
                            for _ in range(5):
                                nc.tensor.matmul(
                                    ps_warm[:], scratch[:], scratch[:],
                                    start=True, stop=True,
                                    skip_group_check=True,
                                )
                        if b + 1 < BPC and gidx in pf_at:
                            j = pf_at[gidx]
                            r0, r1 = PF_BOUNDS[j], PF_BOUNDS[j + 1]
                            dma = nc.gpsimd.dma_start(
                                xtiles[b + 1][:, r0:r1, :],
                                x_d[b + 1, :, r0:r1, :],
                            )
                            _add_dep_helper(
                                dma.ins,
                                cp.ins,
                                sync=True,
                                reason="pace input prefetch vs compute",
                            )
                            dma8 = nc.gpsimd.dma_start(
                                x8tiles[b + 1][:, r0 : min(r1, HP)],
                                x8_d[b + 1, :, r0 : min(r1, HP)],
                            )
                            _add_dep_helper(
                                dma8.ins,
                                cp.ins,
                                sync=True,
                                reason="pace fp8 input prefetch vs compute",
                            )
                        gidx += 1
                    oh += R

    nc.compile()
    return nc


def _get_nc():
    if "nc" not in _CACHE:
        _CACHE["nc"] = _build_nc()
    return _CACHE["nc"]


LAST_RESULT = None


def kernel(x, weight, trace=False):
    global LAST_RESULT
    import ml_dtypes

    x = np.ascontiguousarray(np.asarray(x, dtype=np.float32))
    # Host-side casts: fp16 for the 7 plain taps, e4m3 for the DoubleRow
    # pair. (Values here are well within TRN e4m3's +-240 range, where
    # the OCP e4m3fn bit patterns match TRN FP8_EXP4.)
    x16 = x.astype(np.float16)
    x8 = x.astype(ml_dtypes.float8_e4m3fn)
    # pair-interleave: x8p[..., r, c, i] = x8[..., r+i, c]
    x8p = np.ascontiguousarray(
        np.stack([x8[:, :, 0 : H - 1, :], x8[:, :, 1:H, :]], axis=-1)
    )

    weight = np.asarray(weight, dtype=np.float32)
    wct = weight.reshape(2, 128, CIN, KH, KW)  # [ct, co128, ci, kh, kw]
    # fp16 taps packed [ci, ct, tap7, co128]
    w16 = np.ascontiguousarray(
        np.stack([wct[:, :, :, kh, kw] for (kh, kw) in TAPS16], axis=0)
        .transpose(3, 1, 0, 2)
    ).astype(np.float16)
    # fp8 pair packed [ci, ct, 2, co128]
    w8 = np.ascontiguousarray(
        np.stack([wct[:, :, :, kh, kw] for (kh, kw) in TAPS8], axis=0)
        .transpose(3, 1, 0, 2)
    ).astype(ml_dtypes.float8_e4m3fn)

    nc = _get_nc()
    in_maps = [
        {
            "x": x16[i * BPC : (i + 1) * BPC],
            "x8": x8p[i * BPC : (i + 1) * BPC],
            "w": w16,
            "w8": w8,
        }
        for i in range(NCORES)
    ]
    res = run_bass_kernel_spmd(
        nc, in_maps, core_ids=list(range(NCORES)), trace=trace
    )
    LAST_RESULT = res
    out = np.concatenate([r["o"] for r in res.results], axis=0)
    return out
